# revision 8
# baseline (speedup 1.0000x reference)
"""Causal single-head attention (B=4, S=2048, D=1024, f32) on 8 trn2 cores.

Sharding: data-parallel over batch (4) x 2-way causal-balanced query split.
Core c handles batch b=c//2 and query 128-row blocks {2j+h : j=0..7} where
h=c%2.  Each core computes K/V projections for its whole batch (duplicated
across the pair), Q projection for its own rows, then block-causal
attention.  A per-core column permutation of x^T (own-parity blocks first,
other-parity second) makes the instruction stream identical on all 8 cores;
the residual h-asymmetry (whether the "other parity" boundary block is
fully-masked or fully-valid) is carried by a per-core 128x128 mask input.

All matmuls run in bf16 (inputs pre-cast/pre-transposed on the host), f32
PSUM accumulation, f32 softmax normalization and f32 output.

The final Tile drain / multi-wait instructions are legalized by Bacc's
generate_event_semaphores pass (this container's walrus accepts at most one
sync wait per instruction), so the program is built with bacc.Bacc and
finalized before running.

Scores are computed transposed (s^T[k,q]) so that:
  - exp(scale*s) goes psum -> sbuf on the scalar engine with no max pass
    (|scale*s| <= ~5 for this data, exp is safe in f32/bf16),
  - p^T is directly the stationary operand of the context matmul, and
  - row sums come from ones^T @ p^T matmuls accumulated in PSUM.
"""

import numpy as np
import ml_dtypes

B, S, D = 4, 2048, 1024
P = 128
DI = D // P          # 8 contraction subtiles
NBLK = S // P        # 16 sequence blocks
NSLOT = 8            # query blocks per core
QCORE = NSLOT * P    # 1024 query rows per core
SCALE = 1.0 / 32.0   # 1/sqrt(D)
BF16 = ml_dtypes.bfloat16

_PROGRAM = None


def _slot_kbs(j):
    """Permuted k-block indices slot j attends to (uniform across cores)."""
    return list(range(0, j + 1)) + list(range(NSLOT, NSLOT + j + 1))


def _build_program():
    import concourse.bacc as bacc
    import concourse.mybir as mybir
    import concourse.tile as tile

    dt = mybir.dt
    f32 = dt.float32
    bf = dt.bfloat16
    Exp = mybir.ActivationFunctionType.Exp

    nc = bacc.Bacc("TRN2")
    xT = nc.dram_tensor("xT", [D, S], bf, kind="ExternalInput")
    wqT = nc.dram_tensor("wqT", [D, D], bf, kind="ExternalInput")
    wkT = nc.dram_tensor("wkT", [D, D], bf, kind="ExternalInput")
    wvT = nc.dram_tensor("wvT", [D, D], bf, kind="ExternalInput")
    tri = nc.dram_tensor("tri", [P, P], bf, kind="ExternalInput")
    m2 = nc.dram_tensor("m2", [P, P], bf, kind="ExternalInput")
    y = nc.dram_tensor("y", [QCORE, D], f32, kind="ExternalOutput")

    with tile.TileContext(nc) as tc:
        with tc.tile_pool(name="pers", bufs=1) as pers:
            tri_sb = pers.tile([P, P], bf, tag="tri", name="tri")
            nc.sync.dma_start(tri_sb[:], tri[:])
            m2_sb = pers.tile([P, P], bf, tag="m2", name="m2")
            nc.sync.dma_start(m2_sb[:], m2[:])
            ones_sb = pers.tile([P, 1], bf, tag="ones", name="ones")
            nc.gpsimd.memset(ones_sb[:], 1.0)

            # persistent per-core tensors (bf16): kT [o,s], qT [o,q], v [s,o]
            kT_o = [pers.tile([P, S], bf, tag=f"kT{oi}", name=f"kT{oi}") for oi in range(DI)]
            qT_o = [pers.tile([P, QCORE], bf, tag=f"qT{oi}", name=f"qT{oi}") for oi in range(DI)]
            v_s = [pers.tile([P, D], bf, tag=f"v{si}", name=f"v{si}") for si in range(NBLK)]

            # ---- load + projections (xT / weights freed afterwards) ----
            # Input DMAs are chunked and ordered by first use (wk, then xT
            # 512-col chunks, then wq, wv) and the kT loop iterates s-chunk
            # outer so the PE starts after ~3MB arrives and never stalls on
            # input bandwidth afterwards.  xT chunks load via gpsimd (SWDGE)
            # to run in parallel with the sync-queue weight loads.
            NSC = S // 512
            with (
                tc.tile_pool(name="ld", bufs=1) as ld,
                tc.tile_pool(name="ppsum", bufs=4, space="PSUM") as ppsum,
            ):
                xT4 = xT.rearrange("(di p) (sc s) -> di p sc s", p=P, s=512)
                wq3 = wqT.rearrange("(di p) o -> di p o", p=P)
                wk3 = wkT.rearrange("(di p) o -> di p o", p=P)
                wv3 = wvT.rearrange("(di p) o -> di p o", p=P)
                wk_d = []
                for di in range(DI):
                    t = ld.tile([P, D], bf, tag=f"wk{di}", name=f"wk{di}")
                    nc.sync.dma_start(t[:, 0:512], wk3[di, :, 0:512])
                    wk_d.append(t)
                xT_c = [[None] * NSC for _ in range(DI)]
                for di in range(DI):  # first s-chunk right after the wk halves
                    t = ld.tile([P, 512], bf, tag=f"xT{di}_0", name=f"xT{di}_0")
                    nc.sync.dma_start(t[:], xT4[di, :, 0])
                    xT_c[di][0] = t
                for di in range(DI):
                    nc.sync.dma_start(wk_d[di][:, 512:D], wk3[di, :, 512:D])
                for sc in range(1, NSC):
                    for di in range(DI):
                        t = ld.tile([P, 512], bf, tag=f"xT{di}_{sc}", name=f"xT{di}_{sc}")
                        nc.sync.dma_start(t[:], xT4[di, :, sc])
                        xT_c[di][sc] = t
                wq_d, wv_d = [], []
                for name, src3, dst in (("wq", wq3, wq_d), ("wv", wv3, wv_d)):
                    for di in range(DI):
                        t = ld.tile([P, D], bf, tag=f"{name}{di}", name=f"{name}{di}")
                        nc.gpsimd.dma_start(t[:], src3[di])
                        dst.append(t)

                def proj(lhs_fn, rhs_fn, out_tile, out_cols, n):
                    ps = ppsum.tile([P, n], f32, tag="ppsum", name="ppsum")
                    for di in range(DI):
                        nc.tensor.matmul(
                            ps[:],
                            lhs_fn(di),
                            rhs_fn(di),
                            start=(di == 0),
                            stop=(di == DI - 1),
                        )
                    nc.vector.tensor_copy(out_tile[:, out_cols], ps[:])

                for sc in range(NSC):  # kT: [o, s] for full seq, s-chunk outer
                    cols = slice(sc * 512, (sc + 1) * 512)
                    for oi in range(DI):
                        oc = slice(oi * P, (oi + 1) * P)
                        proj(
                            lambda di, oc=oc: wk_d[di][:, oc],
                            lambda di, sc=sc: xT_c[di][sc][:],
                            kT_o[oi], cols, 512,
                        )
                for qc in range(QCORE // 512):  # qT: [o, q] own rows
                    cols = slice(qc * 512, (qc + 1) * 512)
                    for oi in range(DI):
                        oc = slice(oi * P, (oi + 1) * P)
                        proj(
                            lambda di, oc=oc: wq_d[di][:, oc],
                            lambda di, qc=qc: xT_c[di][qc][:],
                            qT_o[oi], cols, 512,
                        )
                for si in range(NBLK):  # v: [s, o] for full seq
                    sc, lo = si // 4, (si % 4) * P
                    for oh in range(D // 512):
                        cols = slice(oh * 512, (oh + 1) * 512)
                        proj(
                            lambda di, sc=sc, lo=lo: xT_c[di][sc][:, lo : lo + P],
                            lambda di, cols=cols: wv_d[di][:, cols],
                            v_s[si], cols, 512,
                        )

            # ---- attention ----
            pT = [pers.tile([P, QCORE], bf, tag=f"pT{kb}", name=f"pT{kb}") for kb in range(NBLK)]
            with (
                tc.tile_pool(name="spsum", bufs=2, space="PSUM") as spsum,
                tc.tile_pool(name="rpsum", bufs=2, space="PSUM") as rpsum,
                tc.tile_pool(name="cpsum", bufs=4, space="PSUM") as cpsum,
                tc.tile_pool(name="small", bufs=2) as small,
                tc.tile_pool(name="reciprocals", bufs=1) as rpool,
            ):
                for c in range(2):  # per 512 q: scores in 256-wide chunks
                    for c2 in (2 * c, 2 * c + 1):  # slots {2*c2, 2*c2+1}
                        cols = slice(c2 * 256, (c2 + 1) * 256)
                        for kb in _slot_kbs(2 * c2 + 1):
                            kc = slice(kb * P, (kb + 1) * P)
                            ps = spsum.tile([P, 256], f32, tag="spsum", name="spsum")
                            for oi in range(DI):
                                nc.tensor.matmul(
                                    ps[:],
                                    kT_o[oi][:, kc],
                                    qT_o[oi][:, cols],
                                    start=(oi == 0),
                                    stop=(oi == DI - 1),
                                )
                            nc.scalar.activation(
                                pT[kb][:, cols], ps[:], Exp, scale=SCALE
                            )
                        # boundary masks (multiplicative, post-exp)
                        for j in (2 * c2, 2 * c2 + 1):
                            qc = slice(j * P, (j + 1) * P)
                            nc.vector.tensor_mul(pT[j][:, qc], pT[j][:, qc], tri_sb[:])
                            nc.vector.tensor_mul(
                                pT[NSLOT + j][:, qc], pT[NSLOT + j][:, qc], m2_sb[:]
                            )
                    # row sums (pT.T @ ones -> [128,1] psum, q on partitions)
                    # and context: ctx[q, o] = sum_k p^T[k,q] * v[k,o]
                    for j in range(4 * c, 4 * c + 4):
                        qc = slice(j * P, (j + 1) * P)
                        kbs = _slot_kbs(j)
                        rsp = rpsum.tile([P, 1], f32, tag="rsp", name="rsp")
                        for i, kb in enumerate(kbs):
                            nc.tensor.matmul(
                                rsp[:],
                                pT[kb][:, qc],
                                ones_sb[:, 0:1],
                                start=(i == 0),
                                stop=(i == len(kbs) - 1),
                            )
                        recip = rpool.tile([P, 1], f32, tag=f"recip{j}", name=f"recip{j}")
                        nc.vector.reciprocal(recip[:], rsp[:])
                        for oh in range(D // 512):
                            ocols = slice(oh * 512, (oh + 1) * 512)
                            cps = cpsum.tile([P, 512], f32, tag="cpsum", name="cpsum")
                            for i, kb in enumerate(kbs):
                                nc.tensor.matmul(
                                    cps[:],
                                    pT[kb][:, qc],
                                    v_s[kb][:, ocols],
                                    start=(i == 0),
                                    stop=(i == len(kbs) - 1),
                                )
                            ct = small.tile([P, 512], f32, tag="ct", name="ct")
                            nc.vector.tensor_scalar_mul(ct[:], cps[:], recip[:, 0:1])
                            nc.sync.dma_start(y[qc, ocols], ct[:])
    nc.finalize()
    return nc


def _get_program():
    global _PROGRAM
    if _PROGRAM is None:
        _PROGRAM = _build_program()
    return _PROGRAM


def _host_prep(x, Wq, Wk, Wv):
    """Per-core input maps: transposed/cast weights and per-core permuted x^T."""
    tri_np = (np.arange(P)[None, :] >= np.arange(P)[:, None]).astype(BF16)
    masks = {0: np.zeros((P, P), dtype=BF16), 1: np.ones((P, P), dtype=BF16)}
    wqT = np.ascontiguousarray(np.asarray(Wq).T).astype(BF16)
    wkT = np.ascontiguousarray(np.asarray(Wk).T).astype(BF16)
    wvT = np.ascontiguousarray(np.asarray(Wv).T).astype(BF16)
    in_maps = []
    for c in range(8):
        b, h = c // 2, c % 2
        perm = [2 * j + h for j in range(NSLOT)] + [
            2 * j + (1 - h) for j in range(NSLOT)
        ]
        xTb = np.asarray(x[b]).T.reshape(D, NBLK, P)[:, perm, :].reshape(D, S)
        in_maps.append(
            {
                "xT": np.ascontiguousarray(xTb).astype(BF16),
                "wqT": wqT,
                "wkT": wkT,
                "wvT": wvT,
                "tri": tri_np,
                "m2": masks[h],
            }
        )
    return in_maps


def run(x, Wq, Wk, Wv, **spmd_kwargs):
    """Run on all 8 cores; returns (out [B,S,D] f32, BassKernelResults)."""
    from concourse.bass_utils import run_bass_kernel_spmd

    nc = _get_program()
    in_maps = _host_prep(x, Wq, Wk, Wv)
    res = run_bass_kernel_spmd(nc, in_maps, core_ids=list(range(8)), **spmd_kwargs)
    out = np.empty((B, S, D), dtype=np.float32)
    for c in range(8):
        b, h = c // 2, c % 2
        yc = res.results[c]["y"]
        for j in range(NSLOT):
            g = 2 * j + h
            out[b, g * P : (g + 1) * P, :] = yc[j * P : (j + 1) * P, :]
    return out, res


def kernel(x, Wq, Wk, Wv):
    out, _ = run(x, Wq, Wk, Wv)
    return out


# revision 9
# speedup vs baseline: 1.0424x; 1.0424x over previous
"""Causal single-head attention (B=4, S=2048, D=1024, f32) on 8 trn2 cores.

Sharding: data-parallel over batch (4) x 2-way causal-balanced query split.
Core c handles batch b=c//2 and query 128-row blocks {2j+h : j=0..7} where
h=c%2.  Each core computes K/V projections for its whole batch (duplicated
across the pair), Q projection for its own rows, then block-causal
attention.  A per-core column permutation of x^T (own-parity blocks first,
other-parity second) makes the instruction stream identical on all 8 cores;
the residual h-asymmetry (whether the "other parity" boundary block is
fully-masked or fully-valid) is carried by a per-core 128x128 mask input.

All matmuls run in bf16 (inputs pre-cast/pre-transposed on the host), f32
PSUM accumulation, f32 softmax normalization and f32 output.

The final Tile drain / multi-wait instructions are legalized by Bacc's
generate_event_semaphores pass (this container's walrus accepts at most one
sync wait per instruction), so the program is built with bacc.Bacc and
finalized before running.

Scores are computed transposed (s^T[k,q]) so that:
  - exp(scale*s) goes psum -> sbuf on the scalar engine with no max pass
    (|scale*s| <= ~5 for this data, exp is safe in f32/bf16),
  - p^T is directly the stationary operand of the context matmul, and
  - row sums come from ones^T @ p^T matmuls accumulated in PSUM.
"""

import numpy as np
import ml_dtypes

B, S, D = 4, 2048, 1024
P = 128
DI = D // P          # 8 contraction subtiles
NBLK = S // P        # 16 sequence blocks
NSLOT = 8            # query blocks per core
QCORE = NSLOT * P    # 1024 query rows per core
SCALE = 1.0 / 32.0   # 1/sqrt(D)
BF16 = ml_dtypes.bfloat16

_PROGRAM = None


def _slot_kbs(j):
    """Permuted k-block indices slot j attends to (uniform across cores)."""
    return list(range(0, j + 1)) + list(range(NSLOT, NSLOT + j + 1))


def _build_program():
    import concourse.bacc as bacc
    import concourse.mybir as mybir
    import concourse.tile as tile

    dt = mybir.dt
    f32 = dt.float32
    bf = dt.bfloat16
    Exp = mybir.ActivationFunctionType.Exp

    nc = bacc.Bacc("TRN2")
    xT = nc.dram_tensor("xT", [D, S], bf, kind="ExternalInput")
    wqT = nc.dram_tensor("wqT", [D, D], bf, kind="ExternalInput")
    wkT = nc.dram_tensor("wkT", [D, D], bf, kind="ExternalInput")
    wvT = nc.dram_tensor("wvT", [D, D], bf, kind="ExternalInput")
    tri = nc.dram_tensor("tri", [P, P], bf, kind="ExternalInput")
    m2 = nc.dram_tensor("m2", [P, P], bf, kind="ExternalInput")
    y = nc.dram_tensor("y", [QCORE, D], f32, kind="ExternalOutput")

    with tile.TileContext(nc) as tc:
        with tc.tile_pool(name="pers", bufs=1) as pers:
            tri_sb = pers.tile([P, P], bf, tag="tri", name="tri")
            nc.sync.dma_start(tri_sb[:], tri[:])
            m2_sb = pers.tile([P, P], bf, tag="m2", name="m2")
            nc.sync.dma_start(m2_sb[:], m2[:])
            ones_sb = pers.tile([P, 1], bf, tag="ones", name="ones")
            nc.gpsimd.memset(ones_sb[:], 1.0)

            # persistent per-core tensors (bf16): kT [o,s], qT [o,q], v [s,o]
            kT_o = [pers.tile([P, S], bf, tag=f"kT{oi}", name=f"kT{oi}") for oi in range(DI)]
            qT_o = [pers.tile([P, QCORE], bf, tag=f"qT{oi}", name=f"qT{oi}") for oi in range(DI)]
            v_s = [pers.tile([P, D], bf, tag=f"v{si}", name=f"v{si}") for si in range(NBLK)]

            # ---- load + projections (xT / weights freed afterwards) ----
            # Input DMAs are chunked and ordered by first use (wk, then xT
            # 512-col chunks, then wq, wv) and the kT loop iterates s-chunk
            # outer so the PE starts after ~3MB arrives and never stalls on
            # input bandwidth afterwards.  xT chunks load via gpsimd (SWDGE)
            # to run in parallel with the sync-queue weight loads.
            NSC = S // 512
            with (
                tc.tile_pool(name="ld", bufs=1) as ld,
                tc.tile_pool(name="ppsum", bufs=4, space="PSUM") as ppsum,
            ):
                xT4 = xT.rearrange("(di p) (sc s) -> di p sc s", p=P, s=512)
                wq3 = wqT.rearrange("(di p) o -> di p o", p=P)
                wk3 = wkT.rearrange("(di p) o -> di p o", p=P)
                wv3 = wvT.rearrange("(di p) o -> di p o", p=P)
                wk_d = []
                for di in range(DI):
                    t = ld.tile([P, D], bf, tag=f"wk{di}", name=f"wk{di}")
                    nc.sync.dma_start(t[:, 0:512], wk3[di, :, 0:512])
                    wk_d.append(t)
                xT_c = [[None] * NSC for _ in range(DI)]
                for di in range(DI):  # first s-chunk right after the wk halves
                    t = ld.tile([P, 512], bf, tag=f"xT{di}_0", name=f"xT{di}_0")
                    nc.sync.dma_start(t[:], xT4[di, :, 0])
                    xT_c[di][0] = t
                for di in range(DI):
                    nc.sync.dma_start(wk_d[di][:, 512:D], wk3[di, :, 512:D])
                for sc in range(1, NSC):
                    for di in range(DI):
                        t = ld.tile([P, 512], bf, tag=f"xT{di}_{sc}", name=f"xT{di}_{sc}")
                        nc.sync.dma_start(t[:], xT4[di, :, sc])
                        xT_c[di][sc] = t
                wq_d, wv_d = [], []
                for name, src3, dst in (("wq", wq3, wq_d), ("wv", wv3, wv_d)):
                    for di in range(DI):
                        t = ld.tile([P, D], bf, tag=f"{name}{di}", name=f"{name}{di}")
                        nc.sync.dma_start(t[:], src3[di])
                        dst.append(t)

                def proj(lhs_fn, rhs_fn, out_tile, out_cols, n):
                    ps = ppsum.tile([P, n], f32, tag="ppsum", name="ppsum")
                    for di in range(DI):
                        nc.tensor.matmul(
                            ps[:],
                            lhs_fn(di),
                            rhs_fn(di),
                            start=(di == 0),
                            stop=(di == DI - 1),
                        )
                    nc.vector.tensor_copy(out_tile[:, out_cols], ps[:])

                for sc in range(NSC):  # kT: [o, s] for full seq, s-chunk outer
                    cols = slice(sc * 512, (sc + 1) * 512)
                    for oi in range(DI):
                        oc = slice(oi * P, (oi + 1) * P)
                        proj(
                            lambda di, oc=oc: wk_d[di][:, oc],
                            lambda di, sc=sc: xT_c[di][sc][:],
                            kT_o[oi], cols, 512,
                        )
                for qc in range(QCORE // 512):  # qT: [o, q] own rows
                    cols = slice(qc * 512, (qc + 1) * 512)
                    for oi in range(DI):
                        oc = slice(oi * P, (oi + 1) * P)
                        proj(
                            lambda di, oc=oc: wq_d[di][:, oc],
                            lambda di, qc=qc: xT_c[di][qc][:],
                            qT_o[oi], cols, 512,
                        )
                for si in range(NBLK):  # v: [s, o] for full seq
                    sc, lo = si // 4, (si % 4) * P
                    for oh in range(D // 512):
                        cols = slice(oh * 512, (oh + 1) * 512)
                        proj(
                            lambda di, sc=sc, lo=lo: xT_c[di][sc][:, lo : lo + P],
                            lambda di, cols=cols: wv_d[di][:, cols],
                            v_s[si], cols, 512,
                        )

            # ---- attention ----
            pT = [pers.tile([P, QCORE], bf, tag=f"pT{kb}", name=f"pT{kb}") for kb in range(NBLK)]
            with (
                tc.tile_pool(name="spsum", bufs=2, space="PSUM") as spsum,
                tc.tile_pool(name="rpsum", bufs=2, space="PSUM") as rpsum,
                tc.tile_pool(name="cpsum", bufs=4, space="PSUM") as cpsum,
                tc.tile_pool(name="small", bufs=2) as small,
                tc.tile_pool(name="reciprocals", bufs=1) as rpool,
            ):
                for c in range(2):  # per 512 q: scores in 256-wide chunks
                    for c2 in (2 * c, 2 * c + 1):  # slots {2*c2, 2*c2+1}
                        cols = slice(c2 * 256, (c2 + 1) * 256)
                        for kb in _slot_kbs(2 * c2 + 1):
                            kc = slice(kb * P, (kb + 1) * P)
                            ps = spsum.tile([P, 256], f32, tag="spsum", name="spsum")
                            for oi in range(DI):
                                nc.tensor.matmul(
                                    ps[:],
                                    kT_o[oi][:, kc],
                                    qT_o[oi][:, cols],
                                    start=(oi == 0),
                                    stop=(oi == DI - 1),
                                )
                            nc.scalar.activation(
                                pT[kb][:, cols], ps[:], Exp, scale=SCALE
                            )
                        # boundary masks (multiplicative, post-exp)
                        for j in (2 * c2, 2 * c2 + 1):
                            qc = slice(j * P, (j + 1) * P)
                            nc.vector.tensor_mul(pT[j][:, qc], pT[j][:, qc], tri_sb[:])
                            nc.vector.tensor_mul(
                                pT[NSLOT + j][:, qc], pT[NSLOT + j][:, qc], m2_sb[:]
                            )
                    # row sums (pT.T @ ones -> [128,1] psum, q on partitions)
                    # and context: ctx[q, o] = sum_k p^T[k,q] * v[k,o]
                    for j in range(4 * c, 4 * c + 4):
                        qc = slice(j * P, (j + 1) * P)
                        kbs = _slot_kbs(j)
                        rsp = rpsum.tile([P, 1], f32, tag="rsp", name="rsp")
                        for i, kb in enumerate(kbs):
                            nc.tensor.matmul(
                                rsp[:],
                                pT[kb][:, qc],
                                ones_sb[:, 0:1],
                                start=(i == 0),
                                stop=(i == len(kbs) - 1),
                            )
                        recip = rpool.tile([P, 1], f32, tag=f"recip{j}", name=f"recip{j}")
                        nc.vector.reciprocal(recip[:], rsp[:])
                        for oh in range(D // 512):
                            ocols = slice(oh * 512, (oh + 1) * 512)
                            cps = cpsum.tile([P, 512], f32, tag="cpsum", name="cpsum")
                            for i, kb in enumerate(kbs):
                                nc.tensor.matmul(
                                    cps[:],
                                    pT[kb][:, qc],
                                    v_s[kb][:, ocols],
                                    start=(i == 0),
                                    stop=(i == len(kbs) - 1),
                                )
                            ct = small.tile([P, 512], f32, tag="ct", name="ct")
                            nc.vector.tensor_scalar_mul(ct[:], cps[:], recip[:, 0:1])
                            nc.sync.dma_start(y[qc, ocols], ct[:])
    nc.finalize()
    return nc


def _get_program():
    global _PROGRAM
    if _PROGRAM is None:
        _PROGRAM = _build_program()
    return _PROGRAM


def _host_prep(x, Wq, Wk, Wv):
    """Per-core input maps: transposed/cast weights and per-core permuted x^T."""
    tri_np = (np.arange(P)[None, :] >= np.arange(P)[:, None]).astype(BF16)
    masks = {0: np.zeros((P, P), dtype=BF16), 1: np.ones((P, P), dtype=BF16)}
    wqT = np.ascontiguousarray(np.asarray(Wq).T).astype(BF16)
    wkT = np.ascontiguousarray(np.asarray(Wk).T).astype(BF16)
    wvT = np.ascontiguousarray(np.asarray(Wv).T).astype(BF16)
    in_maps = []
    for c in range(8):
        b, h = c // 2, c % 2
        perm = [2 * j + h for j in range(NSLOT)] + [
            2 * j + (1 - h) for j in range(NSLOT)
        ]
        xTb = np.asarray(x[b]).T.reshape(D, NBLK, P)[:, perm, :].reshape(D, S)
        in_maps.append(
            {
                "xT": np.ascontiguousarray(xTb).astype(BF16),
                "wqT": wqT,
                "wkT": wkT,
                "wvT": wvT,
                "tri": tri_np,
                "m2": masks[h],
            }
        )
    return in_maps


def run(x, Wq, Wk, Wv, **spmd_kwargs):
    """Run on all 8 cores; returns (out [B,S,D] f32, BassKernelResults)."""
    from concourse.bass_utils import run_bass_kernel_spmd

    nc = _get_program()
    in_maps = _host_prep(x, Wq, Wk, Wv)
    res = run_bass_kernel_spmd(nc, in_maps, core_ids=list(range(8)), **spmd_kwargs)
    out = np.empty((B, S, D), dtype=np.float32)
    for c in range(8):
        b, h = c // 2, c % 2
        yc = res.results[c]["y"]
        for j in range(NSLOT):
            g = 2 * j + h
            out[b, g * P : (g + 1) * P, :] = yc[j * P : (j + 1) * P, :]
    return out, res


def kernel(x, Wq, Wk, Wv):
    out, _ = run(x, Wq, Wk, Wv)
    return out
